# revision 2
# baseline (speedup 1.0000x reference)
"""HAB (hybrid attention block) kernel for 8 Trainium2 NeuronCores.

Sharding: core c -> image b=c//4, row-band k=c%4 (64 rows of 256).
 - attention: each core computes 5 window-rows (80 windows) of the shifted
   image so its 64 output rows are fully covered locally (no collectives).
 - conv branch: 68-row halo slab with zero-filled out-of-image rows and a
   row mask to reproduce SAME zero-padding across band boundaries.
 - channel attention global pool: per-core partial sums -> host sum ->
   second device phase. Everything heavy runs on the NeuronCores.
"""

import numpy as np
import jax
import jax.numpy as jnp
from jax import lax

B = 2
H = W = 256
C = 192
WS = 16
SHIFT = 8
NH = 6
HD = C // NH
CONV_SCALE = 0.01
EPS = 1e-5
NCORES = 8
BAND = 64          # rows per core
CONV_ROWS = BAND + 4   # 2-row halo each side for the 3x3 conv chain
ATTN_WR = 5        # window-rows computed per core
ATTN_ROWS = ATTN_WR * WS  # 80 rolled rows per core

_CACHE = {}


def _ln(x, g, b):
    mu = jnp.mean(x, -1, keepdims=True)
    var = jnp.mean((x - mu) ** 2, -1, keepdims=True)
    return (x - mu) * lax.rsqrt(var + EPS) * g + b


def _gelu(x):
    return jax.nn.gelu(x, approximate=False)


def _phase1(attn_in, conv_in, row_mask, mask_slab, bias, ln1_g, ln1_b,
            qkv_w, qkv_b, proj_w, proj_b, conv1_w, conv1_b, conv2_w, conv2_b):
    # ---- attention on 80 rolled rows ----
    xn = _ln(attn_in, ln1_g, ln1_b)                      # (80,256,C)
    xw = xn.reshape(ATTN_WR, WS, W // WS, WS, C).transpose(0, 2, 1, 3, 4)
    xw = xw.reshape(-1, WS * WS, C)                      # (80 win, 256, C)
    n = WS * WS
    qkv = (xw @ qkv_w + qkv_b).reshape(-1, n, 3, NH, HD).transpose(2, 0, 3, 1, 4)
    q = qkv[0] * (HD ** -0.5)
    k = qkv[1]
    v = qkv[2]
    attn = jnp.einsum('bhnd,bhmd->bhnm', q, k)
    attn = attn + bias[None] + mask_slab[:, None]
    attn = jax.nn.softmax(attn, axis=-1)
    out = jnp.einsum('bhnm,bhmd->bhnd', attn, v).transpose(0, 2, 1, 3).reshape(-1, n, C)
    out = out @ proj_w + proj_b                          # (80, 256, C)
    aw = out.reshape(ATTN_WR, W // WS, WS, WS, C).transpose(0, 2, 1, 3, 4)
    aw = aw.reshape(ATTN_ROWS, W, C)
    attn_x = aw[SHIFT:SHIFT + BAND]                      # (64,256,C)

    # ---- conv branch on 68-row slab ----
    xc = _ln(conv_in, ln1_g, ln1_b) * row_mask[:, None, None]
    cv = lax.conv_general_dilated(
        xc[None], conv1_w, (1, 1), [(0, 0), (1, 1)],
        dimension_numbers=('NHWC', 'HWIO', 'NHWC'))[0] + conv1_b   # (66,256,64)
    cv = _gelu(cv) * row_mask[1:1 + BAND + 2, None, None]
    cv = lax.conv_general_dilated(
        cv[None], conv2_w, (1, 1), [(0, 0), (1, 1)],
        dimension_numbers=('NHWC', 'HWIO', 'NHWC'))[0] + conv2_b   # (64,256,C)
    partial_pool = jnp.sum(cv, axis=(0, 1))              # (C,)
    return attn_x, cv, partial_pool


def _phase2(resid, attn_x, cv, pooled, ca1_w, ca1_b, ca2_w, ca2_b,
            ln2_g, ln2_b, fc1_w, fc1_b, fc2_w, fc2_b):
    y = jax.nn.relu(pooled @ ca1_w + ca1_b)
    y = jax.nn.sigmoid(y @ ca2_w + ca2_b)                # (C,)
    x2 = resid + attn_x + CONV_SCALE * (cv * y)
    hmid = _gelu(_ln(x2, ln2_g, ln2_b) @ fc1_w + fc1_b)
    return x2 + hmid @ fc2_w + fc2_b


def _get_compiled():
    if 'p1' not in _CACHE:
        devs = jax.devices()[:NCORES]
        _CACHE['devs'] = devs
        _CACHE['p1'] = jax.pmap(
            _phase1, devices=devs,
            in_axes=(0, 0, 0, 0) + (None,) * 11)
        _CACHE['p2'] = jax.pmap(
            _phase2, devices=devs,
            in_axes=(0, 0, 0, 0) + (None,) * 10)
    return _CACHE['devs'], _CACHE['p1'], _CACHE['p2']


def _prep_host(x, rpi_sa, attn_mask, rpb_table):
    xi = np.asarray(x, np.float32).reshape(B, H, W, C)
    xs = np.roll(xi, (-SHIFT, -SHIFT), (1, 2))
    attn_in = np.empty((NCORES, ATTN_ROWS, W, C), np.float32)
    conv_in = np.zeros((NCORES, CONV_ROWS, W, C), np.float32)
    row_mask = np.ones((NCORES, CONV_ROWS), np.float32)
    mask_slab = np.empty((NCORES, ATTN_WR * (W // WS), WS * WS, WS * WS), np.float32)
    am = np.asarray(attn_mask, np.float32)
    for c in range(NCORES):
        b, k = divmod(c, 4)
        r0 = BAND * k
        attn_in[c] = np.take(xs[b], np.arange(r0 - WS, r0 + BAND), axis=0, mode='wrap')
        lo, hi = r0 - 2, r0 + BAND + 2
        slo, shi = max(lo, 0), min(hi, H)
        conv_in[c, slo - lo:shi - lo] = xi[b, slo:shi]
        if lo < 0:
            row_mask[c, :-lo] = 0.0
        if hi > H:
            row_mask[c, H - hi:] = 0.0
        wrs = (np.arange(4 * k - 1, 4 * k + 4) % (H // WS))
        idx = (wrs[:, None] * (W // WS) + np.arange(W // WS)).ravel()
        mask_slab[c] = am[idx]
    bias = np.asarray(rpb_table, np.float32)[
        np.asarray(rpi_sa, np.int64).ravel()
    ].reshape(WS * WS, WS * WS, NH).transpose(2, 0, 1).copy()
    resid = conv_in[:, 2:2 + BAND]
    return attn_in, conv_in, row_mask, mask_slab, bias, resid


def kernel(x, rpi_sa, attn_mask, h, w, ln1_g, ln1_b, qkv_w, qkv_b, rpb_table,
           proj_w, proj_b, conv1_w, conv1_b, conv2_w, conv2_b,
           ca1_w, ca1_b, ca2_w, ca2_b, ln2_g, ln2_b, fc1_w, fc1_b, fc2_w, fc2_b):
    assert (h, w) == (H, W)
    devs, p1, p2 = _get_compiled()
    attn_in, conv_in, row_mask, mask_slab, bias, resid = _prep_host(
        x, rpi_sa, attn_mask, rpb_table)

    f32 = lambda a: np.asarray(a, np.float32)
    attn_x, cv, pp = p1(attn_in, conv_in, row_mask, mask_slab, bias,
                        f32(ln1_g), f32(ln1_b), f32(qkv_w), f32(qkv_b),
                        f32(proj_w), f32(proj_b), f32(conv1_w), f32(conv1_b),
                        f32(conv2_w), f32(conv2_b))
    pp = np.asarray(pp)                                   # (8, C)
    pooled_img = np.stack([pp[:4].sum(0), pp[4:].sum(0)]) / float(H * W)
    pooled_core = pooled_img[np.arange(NCORES) // 4]      # (8, C)

    out = p2(resid, attn_x, cv, pooled_core,
             f32(ca1_w), f32(ca1_b), f32(ca2_w), f32(ca2_b),
             f32(ln2_g), f32(ln2_b), f32(fc1_w), f32(fc1_b),
             f32(fc2_w), f32(fc2_b))
    out = np.asarray(out)                                 # (8, 64, 256, C)
    full = out.reshape(B, 4, BAND, W, C).reshape(B, H, W, C)
    return full.reshape(B, H * W, C).astype(np.float32)


# revision 3
# speedup vs baseline: 686.5989x; 686.5989x over previous
"""HAB (hybrid attention block) kernel for 8 Trainium2 NeuronCores.

Sharding: core c -> image b=c//4, row-band k=c%4 (64 rows of 256).
 - attention: each core computes 5 window-rows (80 windows) of the shifted
   image so its 64 output rows are fully covered locally.
 - conv branch: 68-row halo slab with zero-filled out-of-image rows and a
   row mask to reproduce SAME zero-padding across band boundaries.
 - channel-attention global pool via grouped lax.psum across the 4 cores
   of each image. Single fused device program; matmuls/convs in bf16 with
   fp32 accumulation; LN/softmax/residuals in fp32.
"""

import numpy as np
import jax
import jax.numpy as jnp
from jax import lax

B = 2
H = W = 256
C = 192
WS = 16
SHIFT = 8
NH = 6
HD = C // NH
CONV_SCALE = 0.01
EPS = 1e-5
NCORES = 8
BAND = 64
CONV_ROWS = BAND + 4
ATTN_WR = 5
ATTN_ROWS = ATTN_WR * WS

_CACHE = {}
_BF = jnp.bfloat16
_F32 = jnp.float32


def _ln(x, g, b):
    mu = jnp.mean(x, -1, keepdims=True)
    var = jnp.mean((x - mu) ** 2, -1, keepdims=True)
    return (x - mu) * lax.rsqrt(var + EPS) * g + b


def _gelu(x):
    return jax.nn.gelu(x, approximate=False)


def _mmf32(a, w):
    return jnp.dot(a.astype(_BF), w.astype(_BF), preferred_element_type=_F32)


def _fwd(attn_in, conv_in, row_mask, mask_slab, resid, bias, ln1_g, ln1_b,
         qkv_w, qkv_b, proj_w, proj_b, conv1_w, conv1_b, conv2_w, conv2_b,
         ca1_w, ca1_b, ca2_w, ca2_b, ln2_g, ln2_b, fc1_w, fc1_b, fc2_w, fc2_b):
    n = WS * WS
    # ---- attention on 80 rolled rows ----
    xn = _ln(attn_in, ln1_g, ln1_b)
    xw = xn.reshape(ATTN_WR, WS, W // WS, WS, C).transpose(0, 2, 1, 3, 4)
    xw = xw.reshape(-1, n, C)
    qkv = (_mmf32(xw, qkv_w) + qkv_b).reshape(-1, n, 3, NH, HD)
    qkv = qkv.transpose(2, 0, 3, 1, 4)
    q = (qkv[0] * (HD ** -0.5)).astype(_BF)
    k = qkv[1].astype(_BF)
    v = qkv[2].astype(_BF)
    attn = jnp.einsum('bhnd,bhmd->bhnm', q, k, preferred_element_type=_F32)
    attn = attn + bias[None] + mask_slab[:, None]
    attn = jax.nn.softmax(attn, axis=-1).astype(_BF)
    out = jnp.einsum('bhnm,bhmd->bhnd', attn, v, preferred_element_type=_F32)
    out = out.transpose(0, 2, 1, 3).reshape(-1, n, C)
    out = _mmf32(out, proj_w) + proj_b
    aw = out.reshape(ATTN_WR, W // WS, WS, WS, C).transpose(0, 2, 1, 3, 4)
    attn_x = aw.reshape(ATTN_ROWS, W, C)[SHIFT:SHIFT + BAND]

    # ---- conv branch on 68-row slab ----
    xc = (_ln(conv_in, ln1_g, ln1_b) * row_mask[:, None, None]).astype(_BF)
    cv = lax.conv_general_dilated(
        xc[None], conv1_w.astype(_BF), (1, 1), [(0, 0), (1, 1)],
        dimension_numbers=('NHWC', 'HWIO', 'NHWC'),
        preferred_element_type=_F32)[0] + conv1_b
    cv = (_gelu(cv) * row_mask[1:1 + BAND + 2, None, None]).astype(_BF)
    cv = lax.conv_general_dilated(
        cv[None], conv2_w.astype(_BF), (1, 1), [(0, 0), (1, 1)],
        dimension_numbers=('NHWC', 'HWIO', 'NHWC'),
        preferred_element_type=_F32)[0] + conv2_b
    partial = jnp.sum(cv, axis=(0, 1))
    pooled = lax.psum(partial, 'i',
                      axis_index_groups=[[0, 1, 2, 3], [4, 5, 6, 7]])
    pooled = pooled / float(H * W)
    y = jax.nn.relu(pooled @ ca1_w + ca1_b)
    y = jax.nn.sigmoid(y @ ca2_w + ca2_b)

    # ---- residual + MLP ----
    x2 = resid + attn_x + CONV_SCALE * (cv * y)
    hmid = _gelu(_mmf32(_ln(x2, ln2_g, ln2_b), fc1_w) + fc1_b)
    return x2 + _mmf32(hmid, fc2_w) + fc2_b


def _get_compiled():
    if 'p' not in _CACHE:
        devs = jax.devices()[:NCORES]
        _CACHE['devs'] = devs
        _CACHE['p'] = jax.pmap(
            _fwd, axis_name='i', devices=devs,
            in_axes=(0, 0, 0, 0, 0) + (None,) * 21)
    return _CACHE['devs'], _CACHE['p']


def _prep_host(x, rpi_sa, attn_mask, rpb_table):
    xi = np.asarray(x, np.float32).reshape(B, H, W, C)
    xs = np.roll(xi, (-SHIFT, -SHIFT), (1, 2))
    attn_in = np.empty((NCORES, ATTN_ROWS, W, C), np.float32)
    conv_in = np.zeros((NCORES, CONV_ROWS, W, C), np.float32)
    row_mask = np.ones((NCORES, CONV_ROWS), np.float32)
    mask_slab = np.empty((NCORES, ATTN_WR * (W // WS), WS * WS, WS * WS),
                         np.float32)
    am = np.asarray(attn_mask, np.float32)
    for c in range(NCORES):
        b, k = divmod(c, 4)
        r0 = BAND * k
        attn_in[c] = np.take(xs[b], np.arange(r0 - WS, r0 + BAND),
                             axis=0, mode='wrap')
        lo, hi = r0 - 2, r0 + BAND + 2
        slo, shi = max(lo, 0), min(hi, H)
        conv_in[c, slo - lo:shi - lo] = xi[b, slo:shi]
        if lo < 0:
            row_mask[c, :-lo] = 0.0
        if hi > H:
            row_mask[c, H - hi:] = 0.0
        wrs = (np.arange(4 * k - 1, 4 * k + 4) % (H // WS))
        idx = (wrs[:, None] * (W // WS) + np.arange(W // WS)).ravel()
        mask_slab[c] = am[idx]
    bias = np.asarray(rpb_table, np.float32)[
        np.asarray(rpi_sa, np.int64).ravel()
    ].reshape(WS * WS, WS * WS, NH).transpose(2, 0, 1).copy()
    resid = conv_in[:, 2:2 + BAND].copy()
    return attn_in, conv_in, row_mask, mask_slab, bias, resid


def kernel(x, rpi_sa, attn_mask, h, w, ln1_g, ln1_b, qkv_w, qkv_b, rpb_table,
           proj_w, proj_b, conv1_w, conv1_b, conv2_w, conv2_b,
           ca1_w, ca1_b, ca2_w, ca2_b, ln2_g, ln2_b, fc1_w, fc1_b, fc2_w, fc2_b):
    assert (h, w) == (H, W)
    devs, p = _get_compiled()
    attn_in, conv_in, row_mask, mask_slab, bias, resid = _prep_host(
        x, rpi_sa, attn_mask, rpb_table)
    f32 = lambda a: np.asarray(a, np.float32)
    out = p(attn_in, conv_in, row_mask, mask_slab, resid, bias,
            f32(ln1_g), f32(ln1_b), f32(qkv_w), f32(qkv_b),
            f32(proj_w), f32(proj_b), f32(conv1_w), f32(conv1_b),
            f32(conv2_w), f32(conv2_b), f32(ca1_w), f32(ca1_b),
            f32(ca2_w), f32(ca2_b), f32(ln2_g), f32(ln2_b),
            f32(fc1_w), f32(fc1_b), f32(fc2_w), f32(fc2_b))
    out = np.asarray(out)
    full = out.reshape(B, 4, BAND, W, C).reshape(B, H, W, C)
    return full.reshape(B, H * W, C).astype(np.float32)
